# revision 1
# baseline (speedup 1.0000x reference)
"""Davis-Yin splitting LP solver kernel for Trainium2 (8 NeuronCores, data parallel).

Math per batch item (B=256 total, 32 per core):
  A = [As | I]  (128 x 640),  P = As_inv = pinv(A)  (640 x 128)
  iterate 50x:
    p2 = relu(s)
    t  = (2-a)*p2 - s - a*c
    r  = As @ t[:512] + t[512:] - b          (down-projection, 128)
    u  = As_inv @ r                          (up-projection, 640)
    s  = (s - p2) + t - u
  out = relu(s)

Device layout (per core):
  - State vectors in "column layout": SBUF [128 partitions, nb*5 cols],
    col (b*5+k) holds elements [128k : 128(k+1)) of item b's 640-vector.
  - Down-proj weights: AsT chunks, lhsT_k[dk, m] = As[m, 128k+dk] (4 per item).
  - Up-proj weights: Pinv chunks, lhsT_j[k, d'] = As_inv[128j+d', k] (5 per item).
  - All matvecs are PE matmuls with the matrix as the (self-loading fp32)
    stationary operand and an N=1 moving vector; elementwise work is batched
    across a half-group of items on ACT/DVE so it overlaps PE work.
"""

import numpy as np

import concourse.bass as bass
import concourse.mybir as mybir
from concourse.tile import TileContext
from concourse.bass_utils import run_bass_kernel_spmd

F32 = mybir.dt.float32
AF = mybir.ActivationFunctionType
ALU = mybir.AluOpType

B, M, N = 256, 128, 512
D = M + N  # 640
NCORES = 8
NB = B // NCORES  # 32 items per core
NUM_ITER = 50
ALPHA, TAU, DECAY = 0.05, 1.0, 10.0


def _alphas(num_iter):
    i = np.arange(num_iter, dtype=np.float32)
    base = np.float32(1.0) - i / np.float32(NUM_ITER)
    return (np.float32(ALPHA) * base ** (np.float32(1.0) / np.float32(DECAY))).astype(
        np.float32
    )


def _legalize_waits_json(raw: bytes) -> bytes:
    """Walrus (this revision) accepts at most 1 sync-wait per instruction
    (2 for EventSemaphore), but Tile emits up to 2 on compute instructions.
    Hoist excess waits onto standalone EventSemaphore instructions inserted
    just before the over-subscribed instruction (same engine, so the waits
    still happen-before it in queue order)."""
    import json as _json

    bir = _json.loads(raw)
    ctr = [0]

    def process_block(instrs):
        out = []
        for inst in instrs:
            si = inst.get("sync_info")
            if si:
                waits = si.get("on_wait") or []
                cap = 2 if inst.get("opcode") == "EventSemaphore" else 1
                if len(waits) > cap:
                    extra, keep = waits[:-cap], waits[-cap:]
                    for i in range(0, len(extra), 2):
                        ctr[0] += 1
                        out.append(
                            {
                                "debug": inst.get("debug", 0),
                                "engine": inst["engine"],
                                "ins": [],
                                "name": f"waitfix_{ctr[0]}",
                                "opcode": "EventSemaphore",
                                "outs": [],
                                "sync_info": {
                                    "on_update": [],
                                    "on_wait": extra[i : i + 2],
                                },
                            }
                        )
                    si["on_wait"] = keep
            out.append(inst)
        return out

    def walk(o):
        if isinstance(o, dict):
            for k, v in o.items():
                if k == "instructions" and isinstance(v, list):
                    o[k] = process_block(v)
                else:
                    walk(v)
        elif isinstance(o, list):
            for v in o:
                walk(v)

    walk(bir)
    return _json.dumps(bir).encode()


def _patch_serialization(nc):
    orig = nc.to_json_bytes

    def patched():
        return _legalize_waits_json(orig())

    nc.to_json_bytes = patched
    return nc


def build_program(nb=NB, num_iter=NUM_ITER, nh=4, wdt=F32):
    """Build the per-core Bass program (identical across cores).

    wdt: dtype of the stationary matvec weights (fp32 or bf16). bf16 gets
    single-pass FWL weight loads (~4x faster PE) at ~1e-3 accuracy cost.
    """
    nc = bass.Bass(use_seq_codegen=True, num_swdge_queues=4)
    AsT_d = nc.dram_tensor("AsT", [nb, 4, 128, 128], wdt, kind="ExternalInput")
    Pinv_d = nc.dram_tensor("Pinv", [nb, 5, 128, 128], wdt, kind="ExternalInput")
    c_d = nc.dram_tensor("ccol", [128, nb * 5], F32, kind="ExternalInput")
    b_d = nc.dram_tensor("bcol", [128, nb], F32, kind="ExternalInput")
    out_d = nc.dram_tensor("out", [128, nb * 5], F32, kind="ExternalOutput")

    alphas = _alphas(num_iter)
    hs = nb // nh  # items per half-group

    with TileContext(nc) as tc:
        with (
            tc.tile_pool(name="wpool", bufs=1) as wpool,
            tc.tile_pool(name="spool", bufs=3) as spool,
            tc.tile_pool(name="tpool", bufs=3) as tpool,
            tc.tile_pool(name="ppool", bufs=1, space="PSUM") as ppool,
        ):
            # Per-item weight tiles: item b's first matmul only waits for its
            # own DMA, not the whole 9.5MB load.
            AsT_t, Pinv_t = [], []
            ccol = wpool.tile([128, nb * 5], F32, tag="ccol")
            bcol = wpool.tile([128, nb], F32, tag="bcol")
            nc.sync.dma_start(out=ccol[:], in_=c_d[:])
            nc.sync.dma_start(out=bcol[:], in_=b_d[:])
            for b in range(nb):
                at = wpool.tile([128, 4 * 128], wdt, tag=f"AsT{b}")
                pv = wpool.tile([128, 5 * 128], wdt, tag=f"Pinv{b}")
                # Alternate issuing engines so the loads spread across DMA
                # queues instead of serializing on the sync HWDGE queue.
                eng_a = nc.sync if b % 2 == 0 else nc.gpsimd
                eng_b = nc.gpsimd if b % 2 == 0 else nc.sync
                eng_a.dma_start(
                    out=at[:].rearrange("p (k j) -> p k j", k=4),
                    in_=AsT_d[b].rearrange("k i j -> i k j"),
                )
                eng_b.dma_start(
                    out=pv[:].rearrange("p (k j) -> p k j", k=5),
                    in_=Pinv_d[b].rearrange("k i j -> i k j"),
                )
                AsT_t.append(at)
                Pinv_t.append(pv)

            # Software pipeline: the elementwise "prep" for half h's iteration
            # i+1 (t, t_mm, tsb, w) is emitted right after its s_new, so it
            # runs on DVE/ACT while the PE chews the other halves' matmuls.
            def emit_prep(h, sh, a):
                sl = slice(h * hs * 5, (h + 1) * hs * 5)
                slb = slice(h * hs, (h + 1) * hs)
                p2s = tpool.tile([128, hs * 5], F32, tag=f"p2s{h}")
                mneg = tpool.tile([128, hs * 5], F32, tag=f"mneg{h}")
                q = tpool.tile([128, hs * 5], F32, tag=f"q{h}")
                t = tpool.tile([128, hs * 5], F32, tag=f"t{h}")
                w = tpool.tile([128, hs * 5], F32, tag=f"w{h}")
                tsb = tpool.tile([128, hs], F32, tag=f"tsb{h}")

                # p2s = (2-a)*relu(s) as one fused DVE tensor_scalar (max, mult)
                # — keeps the PE-feeding chain on a single engine.
                nc.vector.tensor_scalar(
                    p2s[:], sh[:], 0.0, 2.0 - a, op0=ALU.max, op1=ALU.mult
                )
                # mneg = relu(-s)  (so s - p2 = -mneg); off critical path -> ACT
                nc.scalar.activation(mneg[:], sh[:], AF.Relu, scale=-1.0)
                # q = a*c + s;  t = p2s - q (bf16 copy for the PE first)
                nc.vector.scalar_tensor_tensor(
                    q[:], ccol[:, sl], a, sh[:], op0=ALU.mult, op1=ALU.add
                )
                if wdt != F32:
                    t_mm = tpool.tile([128, hs * 5], wdt, tag=f"tbf{h}")
                    nc.vector.tensor_sub(t_mm[:], p2s[:], q[:])
                    nc.vector.tensor_sub(t[:], p2s[:], q[:])
                else:
                    nc.vector.tensor_sub(t[:], p2s[:], q[:])
                    t_mm = t
                # tsb = t_slack - b;  w = t - mneg (= s - p2 + t)
                nc.vector.tensor_sub(tsb[:], t[:, 4::5], bcol[:, slb])
                nc.vector.tensor_sub(w[:], t[:], mneg[:])
                return t_mm, tsb, w

            states, preps = [], []
            for h in range(nh):
                sh0 = spool.tile([128, hs * 5], F32, tag=f"state{h}")
                nc.gpsimd.memset(sh0[:], 0.0)
                states.append(sh0)
                preps.append(emit_prep(h, sh0, float(alphas[0])))

            def emit_down(h):
                t_mm = preps[h][0]
                psum_y = ppool.tile([128, hs], F32, tag=f"py{h}")
                for bi in range(hs):
                    bg = h * hs + bi
                    for k in range(4):
                        nc.tensor.matmul(
                            psum_y[:, bi : bi + 1],
                            lhsT=AsT_t[bg][:, k * 128 : (k + 1) * 128],
                            rhs=t_mm[:, bi * 5 + k : bi * 5 + k + 1],
                            start=(k == 0),
                            stop=(k == 3),
                        )
                return psum_y

            def emit_r(h, psum_y):
                # r = y + t_slack - b  (cast to weight dtype fused)
                tsb = preps[h][1]
                r_mm = tpool.tile([128, hs], wdt, tag=f"rbf{h}")
                nc.vector.tensor_add(r_mm[:], psum_y[:], tsb[:])
                return r_mm

            def emit_up(h, r_mm):
                # up-projection: psum_u[:, bi*5+j] = As_inv chunk j @ r
                psum_u = ppool.tile([128, 5 * hs], F32, tag=f"pu{h}")
                for bi in range(hs):
                    bg = h * hs + bi
                    for j in range(5):
                        nc.tensor.matmul(
                            psum_u[:, bi * 5 + j : bi * 5 + j + 1],
                            lhsT=Pinv_t[bg][:, j * 128 : (j + 1) * 128],
                            rhs=r_mm[:, bi : bi + 1],
                            start=True,
                            stop=True,
                        )
                return psum_u

            def emit_snew(h, psum_u, it):
                # s_new = w - u   (single op: psum_u columns match w layout)
                w = preps[h][2]
                s_new = spool.tile([128, hs * 5], F32, tag=f"state{h}")
                nc.vector.tensor_sub(s_new[:], w[:], psum_u[:])
                states[h] = s_new
                if it + 1 < num_iter:
                    preps[h] = emit_prep(h, s_new, float(alphas[it + 1]))

            # Quarters processed in pairs (A,B) as downA,downB,upA,upB: B's
            # down-matmuls hide the PE->DVE->PE r-latency of A, and vice versa.
            for it in range(num_iter):
                for hp in range(0, nh, 2):
                    A, Bq = hp, hp + 1
                    py_a = emit_down(A)
                    r_a = emit_r(A, py_a)
                    py_b = emit_down(Bq)
                    pu_a = emit_up(A, r_a)
                    r_b = emit_r(Bq, py_b)
                    emit_snew(A, pu_a, it)
                    pu_b = emit_up(Bq, r_b)
                    emit_snew(Bq, pu_b, it)

            final = wpool.tile([128, nb * 5], F32, tag="final")
            for h in range(nh):
                nc.scalar.activation(
                    final[:, h * hs * 5 : (h + 1) * hs * 5], states[h][:], AF.Relu
                )
            nc.sync.dma_start(out=out_d[:], in_=final[:])

    return _patch_serialization(nc)


def _prep_core_inputs(c_input, As, bs, As_inv, nb, np_wdt=np.float32):
    """Host-side marshaling of one core's shard into the device layouts."""
    AsT = np.ascontiguousarray(
        As.reshape(nb, 128, 4, 128).transpose(0, 2, 3, 1)
    ).astype(np_wdt)
    Pinv = np.ascontiguousarray(
        As_inv.reshape(nb, 5, 128, 128).transpose(0, 1, 3, 2)
    ).astype(np_wdt)
    ccol = np.ascontiguousarray(
        c_input.reshape(nb, 5, 128).transpose(2, 0, 1).reshape(128, nb * 5),
        dtype=np.float32,
    )
    bcol = np.ascontiguousarray(bs.T, dtype=np.float32)
    return {"AsT": AsT, "Pinv": Pinv, "ccol": ccol, "bcol": bcol}


WEIGHT_DTYPE = "bf16"  # "f32" or "bf16"


def kernel(c_input, As, bs, As_inv, _trace=False, _nc_cache={}):
    import ml_dtypes

    c_input = np.asarray(c_input, dtype=np.float32)
    As = np.asarray(As, dtype=np.float32)
    bs = np.asarray(bs, dtype=np.float32)
    As_inv = np.asarray(As_inv, dtype=np.float32)

    wdt = mybir.dt.bfloat16 if WEIGHT_DTYPE == "bf16" else F32
    np_wdt = ml_dtypes.bfloat16 if WEIGHT_DTYPE == "bf16" else np.float32
    if "nc" not in _nc_cache:
        _nc_cache["nc"] = build_program(wdt=wdt)
    nc = _nc_cache["nc"]

    in_maps = []
    for core in range(NCORES):
        sl = slice(core * NB, (core + 1) * NB)
        in_maps.append(
            _prep_core_inputs(
                c_input[sl], As[sl], bs[sl], As_inv[sl], NB, np_wdt=np_wdt
            )
        )

    res = run_bass_kernel_spmd(nc, in_maps, core_ids=list(range(NCORES)), trace=_trace)

    out = np.empty((B, D), dtype=np.float32)
    for core in range(NCORES):
        oc = res.results[core]["out"]  # [128, NB*5]
        out[core * NB : (core + 1) * NB] = (
            oc.reshape(128, NB, 5).transpose(1, 2, 0).reshape(NB, D)
        )
    if _trace:
        kernel.last_exec_time_ns = res.exec_time_ns
    return out



# revision 3
# speedup vs baseline: 1.1432x; 1.1432x over previous
"""Davis-Yin splitting LP solver kernel for Trainium2 (8 NeuronCores, data parallel).

Math per batch item (B=256 total, 32 per core):
  A = [As | I]  (128 x 640),  P = As_inv = pinv(A)  (640 x 128)
  iterate 50x:
    p2 = relu(s)
    t  = (2-a)*p2 - s - a*c
    r  = As @ t[:512] + t[512:] - b          (down-projection, 128)
    u  = As_inv @ r                          (up-projection, 640)
    s  = (s - p2) + t - u
  out = relu(s)

Device layout (per core):
  - State vectors in "column layout": SBUF [128 partitions, nb*5 cols],
    col (b*5+k) holds elements [128k : 128(k+1)) of item b's 640-vector.
  - Down-proj weights: AsT chunks, lhsT_k[dk, m] = As[m, 128k+dk] (4 per item).
  - Up-proj weights: Pinv chunks, lhsT_j[k, d'] = As_inv[128j+d', k] (5 per item).
  - Weights quantized host-side to fp8 e3m4 with per-tensor pow2 scales
    (halves the FWL LDWEIGHTS stream vs bf16); descale is folded into the
    existing r / s_new elementwise ops as scalar_tensor_tensor immediates.
  - All matvecs are PE matmuls with the matrix as the stationary operand and
    an N=1 moving vector. Per-iter emission order is all down-groups, all
    r's, all up-groups, then snew+prep per group, so the DVE's strict FIFO
    never head-of-line-blocks a ready PE consumer.
"""

import numpy as np

import concourse.bass as bass
import concourse.mybir as mybir
from concourse.tile import TileContext
from concourse.bass_utils import run_bass_kernel_spmd

F32 = mybir.dt.float32
BF16 = mybir.dt.bfloat16
FP8 = mybir.dt.float8e3
AF = mybir.ActivationFunctionType
ALU = mybir.AluOpType

B, M, N = 256, 128, 512
D = M + N  # 640
NCORES = 8
NB = B // NCORES  # 32 items per core
NUM_ITER = 50
ALPHA, TAU, DECAY = 0.05, 1.0, 10.0


def _alphas(num_iter):
    i = np.arange(num_iter, dtype=np.float32)
    base = np.float32(1.0) - i / np.float32(NUM_ITER)
    return (np.float32(ALPHA) * base ** (np.float32(1.0) / np.float32(DECAY))).astype(
        np.float32
    )


def _legalize_waits_json(raw: bytes) -> bytes:
    """Walrus (this revision) accepts at most 1 sync-wait per instruction
    (2 for EventSemaphore), but Tile emits up to 2 on compute instructions.
    Hoist excess waits onto standalone EventSemaphore instructions inserted
    just before the over-subscribed instruction (same engine, so the waits
    still happen-before it in queue order)."""
    import json as _json

    bir = _json.loads(raw)
    ctr = [0]

    def process_block(instrs):
        out = []
        for inst in instrs:
            si = inst.get("sync_info")
            if si:
                waits = si.get("on_wait") or []
                cap = 2 if inst.get("opcode") == "EventSemaphore" else 1
                if len(waits) > cap:
                    extra, keep = waits[:-cap], waits[-cap:]
                    for i in range(0, len(extra), 2):
                        ctr[0] += 1
                        out.append(
                            {
                                "debug": inst.get("debug", 0),
                                "engine": inst["engine"],
                                "ins": [],
                                "name": f"waitfix_{ctr[0]}",
                                "opcode": "EventSemaphore",
                                "outs": [],
                                "sync_info": {
                                    "on_update": [],
                                    "on_wait": extra[i : i + 2],
                                },
                            }
                        )
                    si["on_wait"] = keep
            out.append(inst)
        return out

    def walk(o):
        if isinstance(o, dict):
            for k, v in o.items():
                if k == "instructions" and isinstance(v, list):
                    o[k] = process_block(v)
                else:
                    walk(v)
        elif isinstance(o, list):
            for v in o:
                walk(v)

    walk(bir)
    return _json.dumps(bir).encode()


def _patch_serialization(nc):
    orig = nc.to_json_bytes

    def patched():
        return _legalize_waits_json(orig())

    nc.to_json_bytes = patched
    return nc


def build_program(nb=NB, num_iter=NUM_ITER, nh=4, inv_sa=1.0, inv_sp=1.0):
    """Build the per-core Bass program (identical across cores).

    inv_sa/inv_sp: reciprocal of the pow2 scales the host baked into the fp8
    AsT / Pinv weights; folded into the r and s_new elementwise ops.
    """
    nc = bass.Bass(use_seq_codegen=True, num_swdge_queues=4)
    hs = nb // nh  # items per group
    # Weights land in partition-major contiguous layout: one long row per
    # partition -> group-sized DMAs move 128 x (hs*K*128) contiguous runs.
    AsT_d = nc.dram_tensor("AsT", [128, nb * 4 * 128], FP8, kind="ExternalInput")
    Pinv_d = nc.dram_tensor("Pinv", [128, nb * 5 * 128], FP8, kind="ExternalInput")
    c_d = nc.dram_tensor("ccol", [128, nb * 5], F32, kind="ExternalInput")
    b_d = nc.dram_tensor("bcol", [128, nb], F32, kind="ExternalInput")
    out_d = nc.dram_tensor("out", [128, nb * 5], F32, kind="ExternalOutput")

    alphas = _alphas(num_iter)

    with TileContext(nc) as tc:
        with (
            tc.tile_pool(name="wpool", bufs=1) as wpool,
            tc.tile_pool(name="spool", bufs=3) as spool,
            tc.tile_pool(name="tpool", bufs=3) as tpool,
            tc.tile_pool(name="ppool", bufs=1, space="PSUM") as ppool,
        ):
            ccol = wpool.tile([128, nb * 5], F32, tag="ccol")
            bcol = wpool.tile([128, nb], F32, tag="bcol")
            nc.sync.dma_start(out=ccol[:], in_=c_d[:])
            nc.sync.dma_start(out=bcol[:], in_=b_d[:])
            # Per-group weight tiles, streamed on 3 idle engine queues in PE
            # consumption order (all AsT groups first, then Pinv groups).
            AsT_g, Pinv_g = [], []
            for h in range(nh):
                at = wpool.tile([128, hs * 4 * 128], FP8, tag=f"AsT{h}")
                pv = wpool.tile([128, hs * 5 * 128], FP8, tag=f"Pinv{h}")
                AsT_g.append(at)
                Pinv_g.append(pv)
            qeng = {0: nc.sync, 1: nc.gpsimd, 2: nc.scalar, 3: nc.gpsimd}
            for h in range(nh):
                sl = slice(h * hs * 4 * 128, (h + 1) * hs * 4 * 128)
                qeng[h].dma_start(out=AsT_g[h][:], in_=AsT_d[:, sl])
            for h in range(nh):
                sl = slice(h * hs * 5 * 128, (h + 1) * hs * 5 * 128)
                qeng[h].dma_start(out=Pinv_g[h][:], in_=Pinv_d[:, sl])

            def emit_prep(h, sh, a):
                sl = slice(h * hs * 5, (h + 1) * hs * 5)
                slb = slice(h * hs, (h + 1) * hs)
                p2s = tpool.tile([128, hs * 5], F32, tag=f"p2s{h}")
                mneg = tpool.tile([128, hs * 5], F32, tag=f"mneg{h}")
                q = tpool.tile([128, hs * 5], F32, tag=f"q{h}")
                t = tpool.tile([128, hs * 5], F32, tag=f"t{h}")
                w = tpool.tile([128, hs * 5], F32, tag=f"w{h}")
                tsb = tpool.tile([128, hs], F32, tag=f"tsb{h}")

                # p2s = (2-a)*relu(s) as one fused DVE tensor_scalar (max, mult)
                nc.vector.tensor_scalar(
                    p2s[:], sh[:], 0.0, 2.0 - a, op0=ALU.max, op1=ALU.mult
                )
                # mneg = relu(-s)  (so s - p2 = -mneg); off critical path -> ACT
                nc.scalar.activation(mneg[:], sh[:], AF.Relu, scale=-1.0)
                # q = a*c + s;  t = p2s - q (bf16 copy for the PE first)
                nc.vector.scalar_tensor_tensor(
                    q[:], ccol[:, sl], a, sh[:], op0=ALU.mult, op1=ALU.add
                )
                t_mm = tpool.tile([128, hs * 5], BF16, tag=f"tbf{h}")
                nc.vector.tensor_sub(t_mm[:], p2s[:], q[:])
                nc.vector.tensor_sub(t[:], p2s[:], q[:])
                # tsb = t_slack - b;  w = t - mneg (= s - p2 + t)
                nc.vector.tensor_sub(tsb[:], t[:, 4::5], bcol[:, slb])
                nc.vector.tensor_sub(w[:], t[:], mneg[:])
                return t_mm, tsb, w

            states, preps = [], []
            for h in range(nh):
                sh0 = spool.tile([128, hs * 5], F32, tag=f"state{h}")
                nc.gpsimd.memset(sh0[:], 0.0)
                states.append(sh0)
                preps.append(emit_prep(h, sh0, float(alphas[0])))

            def emit_down(h):
                t_mm = preps[h][0]
                psum_y = ppool.tile([128, hs], F32, tag=f"py{h}")
                for bi in range(hs):
                    base = (bi * 4) * 128
                    for k in range(4):
                        nc.tensor.matmul(
                            psum_y[:, bi : bi + 1],
                            lhsT=AsT_g[h][:, base + k * 128 : base + (k + 1) * 128],
                            rhs=t_mm[:, bi * 5 + k : bi * 5 + k + 1],
                            start=(k == 0),
                            stop=(k == 3),
                        )
                return psum_y

            def emit_r(h, psum_y):
                # r = y/sA + (t_slack - b)  (descale fused, bf16 for the PE)
                tsb = preps[h][1]
                r_mm = tpool.tile([128, hs], BF16, tag=f"rbf{h}")
                nc.vector.scalar_tensor_tensor(
                    r_mm[:], psum_y[:], inv_sa, tsb[:], op0=ALU.mult, op1=ALU.add
                )
                return r_mm

            def emit_up(h, r_mm):
                # up-projection: psum_u[:, bi*5+j] = Pinv chunk j @ r
                psum_u = ppool.tile([128, 5 * hs], F32, tag=f"pu{h}")
                for bi in range(hs):
                    base = (bi * 5) * 128
                    for j in range(5):
                        nc.tensor.matmul(
                            psum_u[:, bi * 5 + j : bi * 5 + j + 1],
                            lhsT=Pinv_g[h][:, base + j * 128 : base + (j + 1) * 128],
                            rhs=r_mm[:, bi : bi + 1],
                            start=True,
                            stop=True,
                        )
                return psum_u

            def emit_snew(h, psum_u, it):
                # s_new = w - u/sP   (descale fused into the one update op)
                w = preps[h][2]
                s_new = spool.tile([128, hs * 5], F32, tag=f"state{h}")
                nc.vector.scalar_tensor_tensor(
                    s_new[:], psum_u[:], -inv_sp, w[:], op0=ALU.mult, op1=ALU.add
                )
                states[h] = s_new
                if it + 1 < num_iter:
                    preps[h] = emit_prep(h, s_new, float(alphas[it + 1]))

            # Per-iter order: d0..d3 | r0..r3 | u0..u3 | snew+prep per group.
            # PE never waits: r_h is queued on DVE before any snew/prep that
            # depends on late-iteration PE work.
            for it in range(num_iter):
                pys = [emit_down(h) for h in range(nh)]
                rs = [emit_r(h, pys[h]) for h in range(nh)]
                pus = [emit_up(h, rs[h]) for h in range(nh)]
                for h in range(nh):
                    emit_snew(h, pus[h], it)

            final = wpool.tile([128, nb * 5], F32, tag="final")
            for h in range(nh):
                nc.scalar.activation(
                    final[:, h * hs * 5 : (h + 1) * hs * 5], states[h][:], AF.Relu
                )
            nc.sync.dma_start(out=out_d[:], in_=final[:])

    return _patch_serialization(nc)


def _pow2_scale(x, target_max):
    am = float(np.abs(x).max())
    if am == 0.0:
        return 1.0
    return float(2.0 ** np.floor(np.log2(target_max / am)))


def _prep_core_inputs(c_input, As, bs, As_inv, nb, sa, sp):
    """Host-side marshaling of one core's shard into the device layouts."""
    import ml_dtypes

    # AsT_all[p, (b*4+k)*128 + m] = As[b, m, 128k+p], fp8 e3m4 scaled by sa
    AsT = np.ascontiguousarray(
        (As * sa).reshape(nb, 128, 4, 128).transpose(3, 0, 2, 1).reshape(128, -1)
    ).astype(ml_dtypes.float8_e3m4)
    # Pinv_all[m, (b*5+j)*128 + q] = As_inv[b, 128j+q, m], scaled by sp
    Pinv = np.ascontiguousarray(
        (As_inv * sp).reshape(nb, 5, 128, 128).transpose(3, 0, 1, 2).reshape(128, -1)
    ).astype(ml_dtypes.float8_e3m4)
    ccol = np.ascontiguousarray(
        c_input.reshape(nb, 5, 128).transpose(2, 0, 1).reshape(128, nb * 5),
        dtype=np.float32,
    )
    bcol = np.ascontiguousarray(bs.T, dtype=np.float32)
    return {"AsT": AsT, "Pinv": Pinv, "ccol": ccol, "bcol": bcol}


def kernel(c_input, As, bs, As_inv, _trace=False, _nc_cache={}):
    c_input = np.asarray(c_input, dtype=np.float32)
    As = np.asarray(As, dtype=np.float32)
    bs = np.asarray(bs, dtype=np.float32)
    As_inv = np.asarray(As_inv, dtype=np.float32)

    # Per-tensor pow2 scales so fp8 e3m4 (max ~15.5, min normal 0.25) sees
    # well-ranged weights; target 12 leaves rounding headroom.
    sa = _pow2_scale(As, 12.0)
    sp = _pow2_scale(As_inv, 12.0)

    key = ("nc", sa, sp)
    if key not in _nc_cache:
        _nc_cache[key] = build_program(inv_sa=1.0 / sa, inv_sp=1.0 / sp)
    nc = _nc_cache[key]

    in_maps = []
    for core in range(NCORES):
        sl = slice(core * NB, (core + 1) * NB)
        in_maps.append(
            _prep_core_inputs(c_input[sl], As[sl], bs[sl], As_inv[sl], NB, sa, sp)
        )

    res = run_bass_kernel_spmd(nc, in_maps, core_ids=list(range(NCORES)), trace=_trace)

    out = np.empty((B, D), dtype=np.float32)
    for core in range(NCORES):
        oc = res.results[core]["out"]  # [128, NB*5]
        out[core * NB : (core + 1) * NB] = (
            oc.reshape(128, NB, 5).transpose(1, 2, 0).reshape(NB, D)
        )
    if _trace:
        kernel.last_exec_time_ns = res.exec_time_ns
    return out


# revision 4
# speedup vs baseline: 1.2569x; 1.0995x over previous
"""Davis-Yin splitting LP solver kernel for Trainium2 (8 NeuronCores, data parallel).

Math per batch item (B=256 total, 32 per core):
  A = [As | I]  (128 x 640),  P = As_inv = pinv(A)  (640 x 128)
  iterate 50x:
    p2 = relu(s)
    t  = (2-a)*p2 - s - a*c
    r  = As @ t[:512] + t[512:] - b          (down-projection, 128)
    u  = As_inv @ r                          (up-projection, 640)
    s  = (s - p2) + t - u
  out = relu(s)

Device layout (per core):
  - State vectors in "column layout": SBUF [128 partitions, nb*5 cols],
    col (b*5+k) holds elements [128k : 128(k+1)) of item b's 640-vector.
  - Down-proj weights: AsT chunks, lhsT_k[dk, m] = As[m, 128k+dk] (4 per item).
  - Up-proj weights: Pinv chunks, lhsT_j[k, d'] = As_inv[128j+d', k] (5 per item).
  - Weights quantized host-side to fp8 e3m4 with per-tensor pow2 scales
    (halves the FWL LDWEIGHTS stream vs bf16); descale is folded into the
    existing r / s_new elementwise ops as scalar_tensor_tensor immediates.
  - All matvecs are PE matmuls with the matrix as the stationary operand and
    an N=1 moving vector. Per-iter emission order is all down-groups, all
    r's, all up-groups, then snew+prep per group, so the DVE's strict FIFO
    never head-of-line-blocks a ready PE consumer.
"""

import numpy as np

import concourse.bass as bass
import concourse.mybir as mybir
from concourse.tile import TileContext
from concourse.bass_utils import run_bass_kernel_spmd

F32 = mybir.dt.float32
BF16 = mybir.dt.bfloat16
FP8 = mybir.dt.float8e3
AF = mybir.ActivationFunctionType
ALU = mybir.AluOpType

B, M, N = 256, 128, 512
D = M + N  # 640
NCORES = 8
NB = B // NCORES  # 32 items per core
NUM_ITER = 50
ALPHA, TAU, DECAY = 0.05, 1.0, 10.0


def _alphas(num_iter):
    i = np.arange(num_iter, dtype=np.float32)
    base = np.float32(1.0) - i / np.float32(NUM_ITER)
    return (np.float32(ALPHA) * base ** (np.float32(1.0) / np.float32(DECAY))).astype(
        np.float32
    )


def _legalize_waits_json(raw: bytes) -> bytes:
    """BIR post-pass, three jobs:

    1. Merge multiple sem-ge-imm waits on the SAME semaphore into one wait
       with the max value (>= is monotone, so this is exact).
    2. Walrus (this revision) accepts at most 1 sync-wait per instruction
       (2 for EventSemaphore); hoist excess waits onto standalone
       EventSemaphore instructions inserted just before (same engine, so
       the waits still happen-before it in queue order).
    3. Strip the per-Matmult PE-semaphore increments that no wait ever
       references, remapping the surviving waits' target values. Sound
       because PE instructions complete in queue order, so waiting on the
       x-th increment == waiting on the x-th kept incrementer.
    """
    import json as _json

    bir = _json.loads(raw)
    ctr = [0]

    def process_block(instrs):
        out = []
        for inst in instrs:
            si = inst.get("sync_info")
            if si:
                waits = si.get("on_wait") or []
                # merge same-sem ge-waits to the max value
                if len(waits) > 1:
                    merged, by_sem = [], {}
                    for w in waits:
                        if w.get("wait_mode") == "sem-ge-imm":
                            k = w.get("id")
                            if k in by_sem:
                                if w["wait_value"] > by_sem[k]["wait_value"]:
                                    by_sem[k]["wait_value"] = w["wait_value"]
                                continue
                            by_sem[k] = w
                        merged.append(w)
                    waits = merged
                    si["on_wait"] = waits
                cap = 2 if inst.get("opcode") == "EventSemaphore" else 1
                if len(waits) > cap:
                    extra, keep = waits[:-cap], waits[-cap:]
                    for i in range(0, len(extra), 2):
                        ctr[0] += 1
                        out.append(
                            {
                                "debug": inst.get("debug", 0),
                                "engine": inst["engine"],
                                "ins": [],
                                "name": f"waitfix_{ctr[0]}",
                                "opcode": "EventSemaphore",
                                "outs": [],
                                "sync_info": {
                                    "on_update": [],
                                    "on_wait": extra[i : i + 2],
                                },
                            }
                        )
                    si["on_wait"] = keep
            out.append(inst)
        return out

    def walk(o):
        if isinstance(o, dict):
            for k, v in o.items():
                if k == "instructions" and isinstance(v, list):
                    o[k] = process_block(v)
                else:
                    walk(v)
        elif isinstance(o, list):
            for v in o:
                walk(v)

    walk(bir)
    _strip_unused_mm_increments(bir)
    return _json.dumps(bir).encode()


def _strip_unused_mm_increments(bir):
    """Remove sem increments from Matmults whose cumulative count no wait
    targets; remap surviving wait values on that semaphore."""
    blocks = []

    def walk(o):
        if isinstance(o, dict):
            for k, v in o.items():
                if k == "instructions" and isinstance(v, list):
                    blocks.append(v)
                else:
                    walk(v)
        elif isinstance(o, list):
            for v in o:
                walk(v)

    walk(bir)
    all_insts = [i for blk in blocks for i in blk]

    # the PE semaphore = the one Matmults increment
    pe_sems = set()
    for i in all_insts:
        if i.get("opcode") == "Matmult":
            for u in (i.get("sync_info") or {}).get("on_update") or []:
                if u.get("update_mode") == "sem-inc":
                    pe_sems.add(u["id"])
    if len(pe_sems) != 1:
        return
    sem = pe_sems.pop()

    # ordered incrementers of that sem (queue order across blocks);
    # bail if any non-Matmult or non-1 increment touches it
    incs = []
    for i in all_insts:
        for u in (i.get("sync_info") or {}).get("on_update") or []:
            if u.get("id") == sem and u.get("sync_type") == "semaphore":
                if i.get("opcode") != "Matmult" or u.get("update_value") != 1:
                    return
                incs.append((i, u))

    # referenced cumulative values
    referenced = set()
    for i in all_insts:
        for w in (i.get("sync_info") or {}).get("on_wait") or []:
            if w.get("id") == sem:
                if w.get("wait_mode") != "sem-ge-imm":
                    return
                referenced.add(w["wait_value"])

    # keep the x-th incrementer iff x referenced; build prefix-kept map
    prefix = [0] * (len(incs) + 1)
    kept = 0
    for x, (inst, upd) in enumerate(incs, start=1):
        if x in referenced:
            kept += 1
        else:
            si = inst["sync_info"]
            si["on_update"] = [u for u in si["on_update"] if u is not upd]
        prefix[x] = kept

    for i in all_insts:
        for w in (i.get("sync_info") or {}).get("on_wait") or []:
            if w.get("id") == sem:
                v = w["wait_value"]
                w["wait_value"] = prefix[min(v, len(incs))]


def _patch_serialization(nc):
    orig = nc.to_json_bytes

    def patched():
        return _legalize_waits_json(orig())

    nc.to_json_bytes = patched
    return nc


def build_program(nb=NB, num_iter=NUM_ITER, nh=4, inv_sa=1.0, inv_sp=1.0):
    """Build the per-core Bass program (identical across cores).

    inv_sa/inv_sp: reciprocal of the pow2 scales the host baked into the fp8
    AsT / Pinv weights; folded into the r and s_new elementwise ops.
    """
    nc = bass.Bass(use_seq_codegen=True, num_swdge_queues=4)
    hs = nb // nh  # items per group
    # Weights land in partition-major contiguous layout: one long row per
    # partition -> group-sized DMAs move 128 x (hs*K*128) contiguous runs.
    AsT_d = nc.dram_tensor("AsT", [128, nb * 4 * 128], FP8, kind="ExternalInput")
    Pinv_d = nc.dram_tensor("Pinv", [128, nb * 5 * 128], FP8, kind="ExternalInput")
    c_d = nc.dram_tensor("ccol", [128, nb * 5], F32, kind="ExternalInput")
    b_d = nc.dram_tensor("bcol", [128, nb], F32, kind="ExternalInput")
    out_d = nc.dram_tensor("out", [128, nb * 5], F32, kind="ExternalOutput")

    alphas = _alphas(num_iter)

    with TileContext(nc) as tc:
        with (
            tc.tile_pool(name="wpool", bufs=1) as wpool,
            tc.tile_pool(name="spool", bufs=3) as spool,
            tc.tile_pool(name="tpool", bufs=3) as tpool,
            tc.tile_pool(name="ppool", bufs=1, space="PSUM") as ppool,
        ):
            ccol = wpool.tile([128, nb * 5], F32, tag="ccol")
            bcol = wpool.tile([128, nb], F32, tag="bcol")
            nc.sync.dma_start(out=ccol[:], in_=c_d[:])
            nc.sync.dma_start(out=bcol[:], in_=b_d[:])
            # Per-group weight tiles, streamed on 3 idle engine queues in PE
            # consumption order (all AsT groups first, then Pinv groups).
            AsT_g, Pinv_g = [], []
            for h in range(nh):
                at = wpool.tile([128, hs * 4 * 128], FP8, tag=f"AsT{h}")
                pv = wpool.tile([128, hs * 5 * 128], FP8, tag=f"Pinv{h}")
                AsT_g.append(at)
                Pinv_g.append(pv)
            qeng = {0: nc.sync, 1: nc.gpsimd, 2: nc.scalar, 3: nc.gpsimd}
            for h in range(nh):
                sl = slice(h * hs * 4 * 128, (h + 1) * hs * 4 * 128)
                qeng[h].dma_start(out=AsT_g[h][:], in_=AsT_d[:, sl])
            for h in range(nh):
                sl = slice(h * hs * 5 * 128, (h + 1) * hs * 5 * 128)
                qeng[h].dma_start(out=Pinv_g[h][:], in_=Pinv_d[:, sl])

            def emit_prep(h, sh, a):
                sl = slice(h * hs * 5, (h + 1) * hs * 5)
                slb = slice(h * hs, (h + 1) * hs)
                p2s = tpool.tile([128, hs * 5], F32, tag=f"p2s{h}")
                mneg = tpool.tile([128, hs * 5], F32, tag=f"mneg{h}")
                q = tpool.tile([128, hs * 5], F32, tag=f"q{h}")
                t = tpool.tile([128, hs * 5], F32, tag=f"t{h}")
                w = tpool.tile([128, hs * 5], F32, tag=f"w{h}")
                tsb = tpool.tile([128, hs], F32, tag=f"tsb{h}")

                # p2s = (2-a)*relu(s) as one fused DVE tensor_scalar (max, mult)
                nc.vector.tensor_scalar(
                    p2s[:], sh[:], 0.0, 2.0 - a, op0=ALU.max, op1=ALU.mult
                )
                # mneg = relu(-s)  (so s - p2 = -mneg); off critical path -> ACT
                nc.scalar.activation(mneg[:], sh[:], AF.Relu, scale=-1.0)
                # q = a*c + s;  t = p2s - q (bf16 copy for the PE first)
                nc.vector.scalar_tensor_tensor(
                    q[:], ccol[:, sl], a, sh[:], op0=ALU.mult, op1=ALU.add
                )
                t_mm = tpool.tile([128, hs * 5], BF16, tag=f"tbf{h}")
                nc.vector.tensor_sub(t_mm[:], p2s[:], q[:])
                nc.vector.tensor_sub(t[:], p2s[:], q[:])
                # tsb = t_slack - b;  w = t - mneg (= s - p2 + t)
                nc.vector.tensor_sub(tsb[:], t[:, 4::5], bcol[:, slb])
                nc.vector.tensor_sub(w[:], t[:], mneg[:])
                return t_mm, tsb, w

            states, preps = [], []
            for h in range(nh):
                sh0 = spool.tile([128, hs * 5], F32, tag=f"state{h}")
                nc.gpsimd.memset(sh0[:], 0.0)
                states.append(sh0)
                preps.append(emit_prep(h, sh0, float(alphas[0])))

            def emit_down(h):
                t_mm = preps[h][0]
                psum_y = ppool.tile([128, hs], F32, tag=f"py{h}")
                for bi in range(hs):
                    base = (bi * 4) * 128
                    for k in range(4):
                        nc.tensor.matmul(
                            psum_y[:, bi : bi + 1],
                            lhsT=AsT_g[h][:, base + k * 128 : base + (k + 1) * 128],
                            rhs=t_mm[:, bi * 5 + k : bi * 5 + k + 1],
                            start=(k == 0),
                            stop=(k == 3),
                        )
                return psum_y

            def emit_r(h, psum_y):
                # r = y/sA + (t_slack - b)  (descale fused, bf16 for the PE)
                tsb = preps[h][1]
                r_mm = tpool.tile([128, hs], BF16, tag=f"rbf{h}")
                nc.vector.scalar_tensor_tensor(
                    r_mm[:], psum_y[:], inv_sa, tsb[:], op0=ALU.mult, op1=ALU.add
                )
                return r_mm

            def emit_up(h, r_mm):
                # up-projection: psum_u[:, bi*5+j] = Pinv chunk j @ r
                psum_u = ppool.tile([128, 5 * hs], F32, tag=f"pu{h}")
                for bi in range(hs):
                    base = (bi * 5) * 128
                    for j in range(5):
                        nc.tensor.matmul(
                            psum_u[:, bi * 5 + j : bi * 5 + j + 1],
                            lhsT=Pinv_g[h][:, base + j * 128 : base + (j + 1) * 128],
                            rhs=r_mm[:, bi : bi + 1],
                            start=True,
                            stop=True,
                        )
                return psum_u

            def emit_snew(h, psum_u, it):
                # s_new = w - u/sP   (descale fused into the one update op)
                w = preps[h][2]
                s_new = spool.tile([128, hs * 5], F32, tag=f"state{h}")
                nc.vector.scalar_tensor_tensor(
                    s_new[:], psum_u[:], -inv_sp, w[:], op0=ALU.mult, op1=ALU.add
                )
                states[h] = s_new
                if it + 1 < num_iter:
                    preps[h] = emit_prep(h, s_new, float(alphas[it + 1]))

            # Per-iter order: d0..d3 | r0..r3 | u0..u3 | snew+prep per group.
            # PE never waits: r_h is queued on DVE before any snew/prep that
            # depends on late-iteration PE work.
            for it in range(num_iter):
                pys = [emit_down(h) for h in range(nh)]
                rs = [emit_r(h, pys[h]) for h in range(nh)]
                pus = [emit_up(h, rs[h]) for h in range(nh)]
                for h in range(nh):
                    emit_snew(h, pus[h], it)

            final = wpool.tile([128, nb * 5], F32, tag="final")
            for h in range(nh):
                nc.scalar.activation(
                    final[:, h * hs * 5 : (h + 1) * hs * 5], states[h][:], AF.Relu
                )
            nc.sync.dma_start(out=out_d[:], in_=final[:])

    return _patch_serialization(nc)


def _pow2_scale(x, target_max):
    am = float(np.abs(x).max())
    if am == 0.0:
        return 1.0
    return float(2.0 ** np.floor(np.log2(target_max / am)))


def _prep_core_inputs(c_input, As, bs, As_inv, nb, sa, sp):
    """Host-side marshaling of one core's shard into the device layouts."""
    import ml_dtypes

    # AsT_all[p, (b*4+k)*128 + m] = As[b, m, 128k+p], fp8 e3m4 scaled by sa
    AsT = np.ascontiguousarray(
        (As * sa).reshape(nb, 128, 4, 128).transpose(3, 0, 2, 1).reshape(128, -1)
    ).astype(ml_dtypes.float8_e3m4)
    # Pinv_all[m, (b*5+j)*128 + q] = As_inv[b, 128j+q, m], scaled by sp
    Pinv = np.ascontiguousarray(
        (As_inv * sp).reshape(nb, 5, 128, 128).transpose(3, 0, 1, 2).reshape(128, -1)
    ).astype(ml_dtypes.float8_e3m4)
    ccol = np.ascontiguousarray(
        c_input.reshape(nb, 5, 128).transpose(2, 0, 1).reshape(128, nb * 5),
        dtype=np.float32,
    )
    bcol = np.ascontiguousarray(bs.T, dtype=np.float32)
    return {"AsT": AsT, "Pinv": Pinv, "ccol": ccol, "bcol": bcol}


def kernel(c_input, As, bs, As_inv, _trace=False, _nc_cache={}):
    c_input = np.asarray(c_input, dtype=np.float32)
    As = np.asarray(As, dtype=np.float32)
    bs = np.asarray(bs, dtype=np.float32)
    As_inv = np.asarray(As_inv, dtype=np.float32)

    # Per-tensor pow2 scales so fp8 e3m4 (max ~15.5, min normal 0.25) sees
    # well-ranged weights; target 12 leaves rounding headroom.
    sa = _pow2_scale(As, 12.0)
    sp = _pow2_scale(As_inv, 12.0)

    key = ("nc", sa, sp)
    if key not in _nc_cache:
        _nc_cache[key] = build_program(inv_sa=1.0 / sa, inv_sp=1.0 / sp)
    nc = _nc_cache[key]

    in_maps = []
    for core in range(NCORES):
        sl = slice(core * NB, (core + 1) * NB)
        in_maps.append(
            _prep_core_inputs(c_input[sl], As[sl], bs[sl], As_inv[sl], NB, sa, sp)
        )

    res = run_bass_kernel_spmd(nc, in_maps, core_ids=list(range(NCORES)), trace=_trace)

    out = np.empty((B, D), dtype=np.float32)
    for core in range(NCORES):
        oc = res.results[core]["out"]  # [128, NB*5]
        out[core * NB : (core + 1) * NB] = (
            oc.reshape(128, NB, 5).transpose(1, 2, 0).reshape(NB, D)
        )
    if _trace:
        kernel.last_exec_time_ns = res.exec_time_ns
    return out
